# revision 7
# baseline (speedup 1.0000x reference)
"""ChebNet (K=2, 2 layers) on 8 Trainium2 NeuronCores.

Reference math:
    deg[r]  = #edges with row==r ;  dinv = deg>0 ? 1/sqrt(max(deg,1)) : 0
    norm_e  = -dinv[row_e] * dinv[col_e]
    layer(x, W0, W1, b) = x@W0 + segment_sum(norm*x[col], row)@W1 + b
    out = log_softmax(layer2(relu(layer1(x))))

Algebraic reordering used here: segment_sum(norm * x[col]) @ W ==
segment_sum(norm * (x@W)[col]) and norm factorizes into per-node scalings:
    u    = dinv * (x @ W1)          # [N, 16] scaled projected features
    s[r] = sum_{e: row=r} u[col_e]  # the only graph-dependent op
    tx1w = -dinv * s
So message passing moves 64B rows instead of 2KB rows.

Distribution: nodes sharded 12500/core, locally relabeled in descending
in-degree order (host-side layout). Each core computes its z/u slab,
AllGathers the small u table (node-major f32, stored in the relabeled
order) into HBM, gathers its in-edges' u rows with indirect DMA into a
degree-paged SBUF layout (virtual row v = page*128 + lane; page slot count
R_st is set per 2-page strip from the actual degrees, so descending order
makes the page padding ~2%), and reduces pages with one affine
vector-engine tensor_reduce per strip straight into the SBUF accumulator.

Host-side work is layout only: CSR sort of edge_index, degree-sorted
relabeling, page/index array construction, shard slicing; the kernel
output rows are permuted back on the host.
"""

import numpy as np

import concourse.bass as bass
import concourse.mybir as mybir
from concourse.bass_utils import run_bass_kernel_spmd
from concourse.masks import make_identity
from concourse.tile import TileContext

# ---- problem constants (hardcoded per contract) ----
N = 100000
F_IN = 500
HID = 16
C = 7
N_CORES = 8
NL = N // N_CORES             # 12500 local rows per core

PAGES = 98                    # 98*128 = 12544 >= NL virtual rows
STRIP = 2                     # pages per strip (shared slot count R_st)
NSTRIPS = PAGES // STRIP
DUMMY = N                     # index of the all-zero table row
F2 = 8                        # layer-2 feature pad (7 -> 8)
KCH = 4                       # 500 = 4*125 contraction chunks
KC = F_IN // KCH
NT = PAGES
FULLP = NL // 128             # 97 full pages
TAILP = NL - FULLP * 128      # 84 rows in the partial page

fp32 = mybir.dt.float32
i32 = mybir.dt.int32

_cache = {}
_ctr = [0]


def _split_sync_waits(nc, max_waits=1):
    """walrus codegen accepts at most one sync wait per instruction; spill
    extras onto NOPs inserted just before, on the same engine."""
    for bass_bb in nc.bb_map.values():
        bb = bass_bb.bb
        new = []
        changed = False
        for inst in bb.instructions:
            si = inst.sync_info
            if si is not None and si.on_wait and len(si.on_wait) > max_waits:
                waits = list(si.on_wait)
                spill, keep = waits[:-max_waits], waits[-max_waits:]
                for i in range(0, len(spill), max_waits):
                    _ctr[0] += 1
                    nop = mybir.InstNoOp(
                        name=f"I-waitspill-{_ctr[0]}",
                        text_hint="waitspill",
                        bass_nofuse=True,
                    )
                    nop.engine = inst.engine
                    nop.sync_info = mybir.SyncInfo(
                        on_wait=spill[i : i + max_waits], on_update=[]
                    )
                    try:
                        nc.register_instruction(nop)
                    except Exception:
                        pass
                    new.append(nop)
                inst.sync_info = mybir.SyncInfo(
                    on_wait=keep, on_update=list(si.on_update or [])
                )
                changed = True
            new.append(inst)
        if changed:
            bb.instructions = new


def _gather_reduce_layer(nc, tc, table_ap, idx_t, s_all, d, strip_rs, cb):
    """Paged gather from table_ap (DRAM [N+1, d], row N all-zero) with
    idx_t (SBUF [128, ncalls] i32); per-strip page reduce into s_all
    (SBUF [128, PAGES*d])."""
    with tc.tile_pool(name=f"mstrip{d}", bufs=2) as mpool:
        for st in range(NSTRIPS):
            r_st = strip_rs[st]
            if r_st == 0:
                nc.vector.memset(
                    s_all[:, st * STRIP * d:(st + 1) * STRIP * d], 0.0)
                continue
            m = mpool.tile([128, STRIP * r_st * d], fp32, tag="m")
            for pg in range(STRIP):
                for s in range(r_st):
                    c = cb[st] + pg * r_st + s
                    nc.gpsimd.indirect_dma_start(
                        out=m[:, (pg * r_st + s) * d:(pg * r_st + s + 1) * d],
                        out_offset=None,
                        in_=table_ap,
                        in_offset=bass.IndirectOffsetOnAxis(
                            ap=idx_t[:, c:c + 1], axis=0),
                    )
            nc.vector.tensor_reduce(
                out=s_all[:, st * STRIP * d:(st + 1) * STRIP * d]
                .rearrange("p (a f) -> p a f", f=d),
                in_=m[:].rearrange("p (a s f) -> p a f s", s=r_st, f=d),
                axis=mybir.AxisListType.X,
                op=mybir.AluOpType.add,
            )


def _slab_to_rows(nc, dram, slab, d):
    """DMA node-major slab [128, PAGES*d] -> DRAM [NL, d] rows
    (row v = page*128 + lane)."""
    nc.sync.dma_start(
        out=dram[:FULLP * 128, :].rearrange("(a p) f -> p a f", p=128),
        in_=slab[:, :FULLP * d].rearrange("p (a f) -> p a f", f=d),
    )
    nc.sync.dma_start(
        out=dram[FULLP * 128:NL, :],
        in_=slab[:TAILP, FULLP * d:(FULLP + 1) * d],
    )


def _build(strip_rs):
    ncalls = int(sum(STRIP * r for r in strip_rs))
    cb = []
    acc = 0
    for r in strip_rs:
        cb.append(acc)
        acc += STRIP * r

    nc = bass.Bass()

    xT = nc.declare_dram_parameter("xT", [F_IN, NL], fp32, isOutput=False)
    w1 = nc.declare_dram_parameter("w1", [F_IN, 2 * HID], fp32, isOutput=False)
    w2 = nc.declare_dram_parameter("w2", [HID, 2 * F2], fp32, isOutput=False)
    b1r = nc.declare_dram_parameter("b1r", [128, HID], fp32, isOutput=False)
    b2r = nc.declare_dram_parameter("b2r", [128, F2], fp32, isOutput=False)
    degp = nc.declare_dram_parameter("degp", [128, PAGES], fp32, isOutput=False)
    idx = nc.declare_dram_parameter("idx", [128, ncalls], i32, isOutput=False)
    y = nc.declare_dram_parameter("y", [NL, C], fp32, isOutput=True)

    u1_loc = nc.dram_tensor("u1_loc", [NL, HID], fp32)
    u2_loc = nc.dram_tensor("u2_loc", [NL, F2], fp32)
    u1_full = nc.dram_tensor("u1_full", [N + 1, HID], fp32, addr_space="Shared")
    u2_full = nc.dram_tensor("u2_full", [N + 1, F2], fp32, addr_space="Shared")

    groups = [list(range(N_CORES))]

    with TileContext(nc) as tc:
        with tc.tile_pool(name="persist", bufs=1) as pp, \
             tc.tile_pool(name="work", bufs=1) as wp, \
             tc.tile_pool(name="psum", bufs=2, space="PSUM") as pspool:

            w1c = pp.tile([KC, KCH * 2 * HID], fp32)
            for k in range(KCH):
                nc.sync.dma_start(
                    out=w1c[:, k * 2 * HID:(k + 1) * 2 * HID],
                    in_=w1[k * KC:(k + 1) * KC, :])
            w2_t = pp.tile([HID, 2 * F2], fp32)
            nc.sync.dma_start(out=w2_t[:], in_=w2[:])
            b1_t = pp.tile([128, HID], fp32)
            nc.sync.dma_start(out=b1_t[:], in_=b1r[:])
            b2_t = pp.tile([128, F2], fp32)
            nc.sync.dma_start(out=b2_t[:], in_=b2r[:])
            idx_t = pp.tile([128, ncalls], i32)
            nc.sync.dma_start(out=idx_t[:], in_=idx[:])
            ident = pp.tile([128, 128], fp32)
            make_identity(nc, ident[:])

            deg_t = wp.tile([128, PAGES], fp32)
            nc.sync.dma_start(out=deg_t[:], in_=degp[:])
            dinv = pp.tile([128, PAGES], fp32)
            mx1 = wp.tile([128, PAGES], fp32)
            nc.vector.tensor_scalar(out=mx1[:], in0=deg_t[:], scalar1=1.0,
                                    scalar2=None, op0=mybir.AluOpType.max)
            nc.vector.reciprocal(out=mx1[:], in_=mx1[:])
            nc.scalar.activation(out=mx1[:], in_=mx1[:],
                                 func=mybir.ActivationFunctionType.Sqrt)
            gz = wp.tile([128, PAGES], fp32)
            nc.vector.tensor_scalar(out=gz[:], in0=deg_t[:], scalar1=0.0,
                                    scalar2=None, op0=mybir.AluOpType.is_gt)
            nc.vector.tensor_tensor(out=dinv[:], in0=mx1[:], in1=gz[:],
                                    op=mybir.AluOpType.mult)

            zrow = wp.tile([1, HID], fp32)
            nc.vector.memset(zrow[:], 0.0)
            nc.sync.dma_start(out=u1_full[N:N + 1, :], in_=zrow[:])
            zrow2 = wp.tile([1, F2], fp32)
            nc.vector.memset(zrow2[:], 0.0)
            nc.sync.dma_start(out=u2_full[N:N + 1, :], in_=zrow2[:])

            # ---- phase 1: [z1 | u1] = x @ [W1_0 W1_1], u1 scaled by dinv ----
            z1 = pp.tile([128, PAGES * HID], fp32)
            u1 = pp.tile([128, PAGES * HID], fp32)
            with tc.tile_pool(name="xload", bufs=3) as xp:
                for t in range(NT):
                    nlo = t * 128
                    nn = min(128, NL - nlo)
                    if nn <= 0:
                        nc.vector.memset(z1[:, t * HID:(t + 1) * HID], 0.0)
                        nc.vector.memset(u1[:, t * HID:(t + 1) * HID], 0.0)
                        continue
                    xt = xp.tile([KC, KCH * 128], fp32, tag="xt")
                    for k in range(KCH):
                        nc.sync.dma_start(
                            out=xt[:, k * 128:k * 128 + nn],
                            in_=xT[k * KC:(k + 1) * KC, nlo:nlo + nn])
                    ps = pspool.tile([128, 2 * HID], fp32, tag="mm1")
                    for k in range(KCH):
                        nc.tensor.matmul(
                            out=ps[:nn, :],
                            lhsT=xt[:, k * 128:k * 128 + nn],
                            rhs=w1c[:, k * 2 * HID:(k + 1) * 2 * HID],
                            start=(k == 0), stop=(k == KCH - 1),
                        )
                    if nn < 128:
                        nc.vector.memset(z1[:, t * HID:(t + 1) * HID], 0.0)
                        nc.vector.memset(u1[:, t * HID:(t + 1) * HID], 0.0)
                    nc.vector.tensor_copy(
                        out=z1[:nn, t * HID:(t + 1) * HID], in_=ps[:nn, :HID])
                    nc.vector.tensor_tensor(
                        out=u1[:nn, t * HID:(t + 1) * HID],
                        in0=ps[:nn, HID:2 * HID],
                        in1=dinv[:nn, t:t + 1].to_broadcast([nn, HID]),
                        op=mybir.AluOpType.mult)
            _slab_to_rows(nc, u1_loc, u1, HID)
            nc.gpsimd.collective_compute(
                "AllGather", mybir.AluOpType.bypass,
                replica_groups=groups,
                ins=[u1_loc[:, :].opt()],
                outs=[u1_full[:N, :].opt()],
            )

            s1 = pp.tile([128, PAGES * HID], fp32)
            _gather_reduce_layer(nc, tc, u1_full[:, :], idx_t, s1, HID,
                                 strip_rs, cb)

            # ---- combine -> h = relu(z1 - dinv*s1 + b1) ----
            h = pp.tile([128, PAGES * HID], fp32)
            sc = wp.tile([128, PAGES * HID], fp32)
            nc.vector.tensor_tensor(
                out=sc[:].rearrange("p (a f) -> p a f", f=HID),
                in0=s1[:].rearrange("p (a f) -> p a f", f=HID),
                in1=dinv[:].unsqueeze(-1).to_broadcast([128, PAGES, HID]),
                op=mybir.AluOpType.mult)
            nc.vector.tensor_tensor(out=h[:], in0=z1[:], in1=sc[:],
                                    op=mybir.AluOpType.subtract)
            nc.vector.tensor_tensor(
                out=h[:].rearrange("p (a f) -> p a f", f=HID),
                in0=h[:].rearrange("p (a f) -> p a f", f=HID),
                in1=b1_t[:].unsqueeze(1).to_broadcast([128, PAGES, HID]),
                op=mybir.AluOpType.add)
            nc.vector.tensor_scalar(out=h[:], in0=h[:], scalar1=0.0,
                                    scalar2=None, op0=mybir.AluOpType.max)

            # ---- phase 2: [z2 | u2] = h @ [W2_0 W2_1] ----
            z2 = pp.tile([128, PAGES * F2], fp32)
            u2 = pp.tile([128, PAGES * F2], fp32)
            with tc.tile_pool(name="hT", bufs=3) as hp:
                for t in range(NT):
                    ps_t = pspool.tile([HID, 128], fp32, tag="tr")
                    nc.tensor.transpose(
                        out=ps_t[:],
                        in_=h[:, t * HID:(t + 1) * HID],
                        identity=ident[:],
                    )
                    hT = hp.tile([HID, 128], fp32, tag="hTt")
                    nc.vector.tensor_copy(out=hT[:], in_=ps_t[:])
                    ps2 = pspool.tile([128, 2 * F2], fp32, tag="mm2")
                    nc.tensor.matmul(out=ps2[:], lhsT=hT[:], rhs=w2_t[:],
                                     start=True, stop=True)
                    nc.vector.tensor_copy(
                        out=z2[:, t * F2:(t + 1) * F2], in_=ps2[:, :F2])
                    nc.vector.tensor_tensor(
                        out=u2[:, t * F2:(t + 1) * F2],
                        in0=ps2[:, F2:2 * F2],
                        in1=dinv[:, t:t + 1].to_broadcast([128, F2]),
                        op=mybir.AluOpType.mult)
            _slab_to_rows(nc, u2_loc, u2, F2)
            nc.gpsimd.collective_compute(
                "AllGather", mybir.AluOpType.bypass,
                replica_groups=groups,
                ins=[u2_loc[:, :].opt()],
                outs=[u2_full[:N, :].opt()],
            )

            s2 = pp.tile([128, PAGES * F2], fp32)
            _gather_reduce_layer(nc, tc, u2_full[:, :], idx_t, s2, F2,
                                 strip_rs, cb)

            # ---- combine -> logits -> log_softmax ----
            lg = wp.tile([128, PAGES * F2], fp32)
            nc.vector.tensor_tensor(
                out=lg[:].rearrange("p (a f) -> p a f", f=F2),
                in0=s2[:].rearrange("p (a f) -> p a f", f=F2),
                in1=dinv[:].unsqueeze(-1).to_broadcast([128, PAGES, F2]),
                op=mybir.AluOpType.mult)
            nc.vector.tensor_tensor(out=lg[:], in0=z2[:], in1=lg[:],
                                    op=mybir.AluOpType.subtract)
            nc.vector.tensor_tensor(
                out=lg[:].rearrange("p (a f) -> p a f", f=F2),
                in0=lg[:].rearrange("p (a f) -> p a f", f=F2),
                in1=b2_t[:].unsqueeze(1).to_broadcast([128, PAGES, F2]),
                op=mybir.AluOpType.add)

            lgv = lg[:].rearrange("p (a f) -> p a f", f=F2)
            mxr = wp.tile([128, PAGES], fp32)
            nc.vector.tensor_reduce(
                out=mxr[:].unsqueeze(-1),
                in_=lgv[:, :, :C],
                axis=mybir.AxisListType.X, op=mybir.AluOpType.max)
            d0 = wp.tile([128, PAGES * F2], fp32)
            nc.vector.tensor_tensor(
                out=d0[:].rearrange("p (a f) -> p a f", f=F2),
                in0=lgv,
                in1=mxr[:].unsqueeze(-1).to_broadcast([128, PAGES, F2]),
                op=mybir.AluOpType.subtract)
            ex = wp.tile([128, PAGES * F2], fp32)
            nc.scalar.activation(out=ex[:], in_=d0[:],
                                 func=mybir.ActivationFunctionType.Exp)
            sm = wp.tile([128, PAGES], fp32)
            nc.vector.tensor_reduce(
                out=sm[:].unsqueeze(-1),
                in_=ex[:].rearrange("p (a f) -> p a f", f=F2)[:, :, :C],
                axis=mybir.AxisListType.X, op=mybir.AluOpType.add)
            nc.scalar.activation(out=sm[:], in_=sm[:],
                                 func=mybir.ActivationFunctionType.Ln)
            res = wp.tile([128, PAGES * F2], fp32)
            nc.vector.tensor_tensor(
                out=res[:].rearrange("p (a f) -> p a f", f=F2),
                in0=d0[:].rearrange("p (a f) -> p a f", f=F2),
                in1=sm[:].unsqueeze(-1).to_broadcast([128, PAGES, F2]),
                op=mybir.AluOpType.subtract)

            resv = res[:].rearrange("p (a f) -> p a f", f=F2)
            nc.sync.dma_start(
                out=y[:FULLP * 128, :].rearrange("(a p) f -> p a f", p=128),
                in_=resv[:, :FULLP, :C])
            nc.sync.dma_start(
                out=y[FULLP * 128:NL, :],
                in_=resv[:TAILP, FULLP:FULLP + 1, :C].squeeze(1))
    _split_sync_waits(nc)
    return nc


# --------------------------------------------------------------------------
# host-side sharding / layout prep (pure data layout)
# --------------------------------------------------------------------------
def _prep(x, edge_index, W1_0, W1_1, b1, W2_0, W2_1, b2):
    x = np.asarray(x, np.float32)
    ei = np.asarray(edge_index)
    row, col = ei[0].astype(np.int64), ei[1].astype(np.int64)

    order_e = np.argsort(row, kind="stable")
    row_s, col_s = row[order_e], col[order_e]
    deg_full = np.bincount(row_s, minlength=N).astype(np.int64)
    row_ptr = np.zeros(N + 1, np.int64)
    np.cumsum(deg_full, out=row_ptr[1:])

    # per-core descending-degree relabeling + global position map
    orders = []
    newpos = np.empty(N + 1, np.int64)
    newpos[N] = N
    deg_sorted_all = []
    for cidx in range(N_CORES):
        lo = cidx * NL
        deg_l = deg_full[lo:lo + NL]
        o = np.argsort(-deg_l, kind="stable")
        orders.append(o)
        newpos[lo + o] = lo + np.arange(NL)
        ds = np.zeros(PAGES * 128, np.int64)
        ds[:NL] = deg_l[o]
        deg_sorted_all.append(ds)

    # common per-strip slot counts (max over cores)
    strip_rs = []
    for st in range(NSTRIPS):
        r = 0
        for ds in deg_sorted_all:
            r = max(r, int(ds[st * STRIP * 128:(st + 1) * STRIP * 128].max()))
        strip_rs.append(r)
    strip_rs = tuple(strip_rs)
    cb = np.zeros(NSTRIPS, np.int64)
    acc = 0
    for st in range(NSTRIPS):
        cb[st] = acc
        acc += STRIP * strip_rs[st]
    ncalls = int(acc)
    # column base for page pg = cb[pg//STRIP] + (pg%STRIP)*r_st
    colbase = np.array([cb[pg // STRIP] +
                        (pg % STRIP) * strip_rs[pg // STRIP]
                        for pg in range(PAGES)], np.int64)

    xT = np.ascontiguousarray(x.T)
    w1cat = np.concatenate([np.asarray(W1_0, np.float32),
                            np.asarray(W1_1, np.float32)], axis=1)
    w2cat = np.zeros((HID, 2 * F2), np.float32)
    w2cat[:, :C] = np.asarray(W2_0, np.float32)
    w2cat[:, F2:F2 + C] = np.asarray(W2_1, np.float32)
    b1rep = np.tile(np.asarray(b1, np.float32)[None, :], (128, 1))
    b2rep = np.zeros((128, F2), np.float32)
    b2rep[:, :C] = np.asarray(b2, np.float32)[None, :]
    col_mapped = newpos[col_s].astype(np.int32)

    in_maps = []
    for cidx in range(N_CORES):
        lo = cidx * NL
        o = orders[cidx]
        ds = deg_sorted_all[cidx][:NL]  # degree of virtual row v

        idx_arr = np.full((128, ncalls), DUMMY, np.int32)
        vs = np.arange(NL, dtype=np.int64)
        tot = int(ds.sum())
        reps = np.repeat(vs, ds)
        cum = np.cumsum(ds) - ds
        offs = np.arange(tot, dtype=np.int64) - np.repeat(cum, ds)
        src = row_ptr[lo + o[reps]] + offs
        lanes = reps % 128
        cols_pos = colbase[reps // 128] + offs
        idx_arr[lanes, cols_pos] = col_mapped[src]

        degp = np.zeros((128, PAGES), np.float32)
        degp[vs % 128, vs // 128] = ds

        in_maps.append(dict(
            xT=np.ascontiguousarray(xT[:, lo:lo + NL][:, o]),
            w1=w1cat, w2=w2cat, b1r=b1rep, b2r=b2rep,
            degp=degp, idx=idx_arr,
        ))
    return in_maps, strip_rs, orders


def kernel(x, edge_index, W1_0, W1_1, b1, W2_0, W2_1, b2):
    in_maps, strip_rs, orders = _prep(x, edge_index, W1_0, W1_1, b1,
                                      W2_0, W2_1, b2)
    if strip_rs not in _cache:
        _cache[strip_rs] = _build(strip_rs)
    nc = _cache[strip_rs]
    res = run_bass_kernel_spmd(nc, in_maps, list(range(N_CORES)))
    out = np.empty((N, C), np.float32)
    for i in range(N_CORES):
        yv = res.results[i]["y"]
        out[i * NL + orders[i]] = yv
    return out
